# revision 15
# baseline (speedup 1.0000x reference)
"""Fused FADocker coordinate-update kernel for 8 Trainium2 NeuronCores (v3).

Per core c: batch b=c//4, j-range (c%4)*96..+96,
    S'[(a,c'), i] = sum_{j,h} W_j[h,(a,c')] * relu(hwT[h,i] + huT[h,j])
with hw = W_x(h), hu = U_x(h) projections and W_j[h,ac] = TxT[h,a]*Xm4[j,ac]
precomputed on host. Host sums the 8 partials and applies the Tx_b
correction, divide, clip, residual add.

Engine plan (cost-model-driven): H=256 contracts as two 128-row halves.
Each j is a pair job:
  - bf16 (DVE): two bf16 relu tiles (160ns each, 4x DVE mode) -> two bf16
    matmuls (160ns at full PE p-state).
  - fp8 (DVE 260ns/half, ACT 505, Pool 628): both halves of a [128,2,384]
    fp8e4 pair tile -> ONE DoubleRow matmul (80ns) contracting both halves.
fp8 W is error-feedback quantized over j on host so the j-sum telescopes
quantization error (rel err ~3e-3 vs 2.5e-2 naive). All four engines are
balanced near 20.5us; consts load outside the timing loop and PSUM/out
tiles are double-buffered so loop iterations pipeline.
"""

import contextlib
from concurrent.futures import ThreadPoolExecutor

import numpy as np
import ml_dtypes

import concourse.bass as bass
import concourse.tile as tile
from concourse import bacc, mybir
from concourse.bass import RegisterHandles, make_scalar_value
from concourse.bass_utils import run_bass_kernel_spmd

B, L, H, A = 2, 384, 256, 14
NCORES = 8
JSHARD = L // 4          # 96 j's per core
AC = A * 4               # 56 = (a, c') columns
ACP = 64                 # AC padded to 64 (DoubleRow needs k-tile stride %16==0)
P = 128

F32 = mybir.dt.float32
BF16 = mybir.dt.bfloat16
FP8 = mybir.dt.float8e4

# pairs per (engine, dtype): db/df = DVE bf16/fp8, ab/af = ACT, pb/pf = Pool;
# sum = 96
SPLIT = (65, 0, 0, 31, 0, 0)
NFILL = 0                # PE warm-up filler matmuls (first-iteration only aid)
CB = 16                  # bf16 W chunk: j's per DMA
CF = 24                  # fp8 W chunk: j's per DMA

_cached = {}


def assign_jobs(split=None, nclust=6):
    """Per-job engine tags (db/df/ab/af/pb/pf). PE mode switches
    (plain<->DoubleRow) cost ~280ns, but one big DR block serializes after
    the DVE-paced bf16 stream; compromise: nclust clusters, each
    [bf16 chunk, DR chunk], so switches stay few AND the DR work
    interleaves into PE's slack."""
    split = split or SPLIT
    bf = []
    f8 = []
    for tag, n in zip(("db", "ab", "pb", "df", "af", "pf"), (
            split[0], split[2], split[4], split[1], split[3], split[5])):
        lst = bf if tag.endswith("b") else f8
        lst += [tag] * n
    jobs = []
    for c in range(nclust):
        jobs += bf[len(bf) * c // nclust:len(bf) * (c + 1) // nclust]
        jobs += f8[len(f8) * c // nclust:len(f8) * (c + 1) // nclust]
    return jobs


def _build_program(reps=1, dyn_loop=False, split=None, nfill=None):
    split = split or SPLIT
    nfill = NFILL if nfill is None else nfill
    key = ("nc", reps, dyn_loop, split, nfill)
    if key in _cached:
        return _cached[key]

    jobs = assign_jobs(split)
    nb = sum(1 for t in jobs if t.endswith("b"))
    nf = len(jobs) - nb
    nb_chunks = max(1, (nb + CB - 1) // CB)
    nf_chunks = max(1, (nf + CF - 1) // CF)

    nc = bacc.Bacc("TRN2", target_bir_lowering=False, debug=False)

    hw_d = nc.dram_tensor("hw", [2, P, L], BF16, kind="ExternalInput").ap()
    hu_d = nc.dram_tensor("hu", [2, P, JSHARD], F32,
                          kind="ExternalInput").ap()
    Wb_d = nc.dram_tensor("Wb", [P, max(nb, 1) * 2 * ACP], BF16,
                          kind="ExternalInput").ap()
    Wf_d = nc.dram_tensor("Wf", [P, max(nf, 1) * 2 * ACP], FP8,
                          kind="ExternalInput").ap()
    if dyn_loop:
        ln_d = nc.dram_tensor("ln", [1, 1], mybir.dt.int32,
                              kind="ExternalInput").ap()
    Sp_d = nc.dram_tensor("Sp", [AC, L], F32, kind="ExternalOutput").ap()

    with tile.TileContext(nc, trace_sim=False) as tc:
        with (
            tc.tile_pool(name="const", bufs=1) as cpool,
            tc.tile_pool(name="outp", bufs=2) as opool,
            tc.tile_pool(name="rb", bufs=48) as rpool,
            tc.tile_pool(name="fd", bufs=8) as fpool_d,
            tc.tile_pool(name="fa", bufs=70) as fpool_a,
            tc.tile_pool(name="fp", bufs=4) as fpool_p,
            tc.tile_pool(name="psum_s", bufs=2, space="PSUM") as pspool_s,
            tc.tile_pool(name="psum_j", bufs=1, space="PSUM") as pspool_j,
        ):
            # Input tiles: hw/hu first (gate the producers), W chunks in
            # first-use order alternating between the sync HWDGE queue and
            # the scalar SWDGE queue.
            hwe_sb = {}
            hue_sb = {}
            for e in "dap":
                hwe_sb[e] = []
                hue_sb[e] = []
                for k in range(2):
                    eng = nc.sync if (k % 2 == 0) else nc.scalar
                    t = cpool.tile([P, L], BF16, tag=f"hw{e}{k}")
                    eng.dma_start(t[:], hw_d[k])
                    hwe_sb[e].append(t)
                    t2 = cpool.tile([P, JSHARD], F32, tag=f"hu{e}{k}")
                    eng.dma_start(t2[:], hu_d[k])
                    hue_sb[e].append(t2)
            hwT_sb = hwe_sb["d"]
            huT_sb = hue_sb["d"]

            buse = {}
            fuse = {}
            kb = kf = 0
            for pos, t in enumerate(jobs):
                if t.endswith("b"):
                    buse.setdefault(kb // CB, pos)
                    kb += 1
                else:
                    fuse.setdefault(kf // CF, pos)
                    kf += 1
            order = sorted(
                [("b", ck, p) for ck, p in buse.items()]
                + [("f", ck, p) for ck, p in fuse.items()], key=lambda x: x[2])
            wb_sb = [None] * nb_chunks
            wf_sb = [None] * nf_chunks
            for i, (kind, ck, _) in enumerate(order):
                eng = nc.sync if i % 2 == 0 else nc.scalar
                if kind == "b":
                    n_j = min(CB, nb - ck * CB)
                    t = cpool.tile([P, n_j, 2, ACP], BF16, tag=f"wb{ck}")
                    c0 = ck * CB * 2 * ACP
                    eng.dma_start(t[:], Wb_d[:, c0:c0 + n_j * 2 * ACP])
                    wb_sb[ck] = t
                else:
                    n_j = min(CF, nf - ck * CF)
                    t = cpool.tile([P, n_j, 2, ACP], FP8, tag=f"wf{ck}")
                    c0 = ck * CF * 2 * ACP
                    eng.dma_start(t[:], Wf_d[:, c0:c0 + n_j * 2 * ACP])
                    wf_sb[ck] = t

            if dyn_loop:
                ln_t = cpool.tile([1, 1], mybir.dt.int32, tag="ln")
                nc.sync.dma_start(ln_t[:], ln_d[:])
                regs = []
                for e in mybir.ALL_ENGINES:
                    r = nc.alloc_register(e, f"lnreg_{e.name}")
                    nc.engines[e].reg_load(r, ln_t[0:1, 0:1])
                    regs.append(r)
                end_val = make_scalar_value(RegisterHandles(regs),
                                            min_val=0, max_val=1 << 20)
                loop_cm = tc.For_i(0, end_val, 1)
            else:
                loop_cm = contextlib.nullcontext()
            with loop_cm:
                for rep in range(reps):
                    _emit_body(nc, tc, jobs, nfill, hwe_sb, hue_sb, wb_sb,
                               wf_sb, Sp_d, opool, rpool, fpool_d, fpool_a,
                               fpool_p, pspool_s, pspool_j)

    nc.compile()
    _cached[key] = nc
    return nc


def _emit_body(nc, tc, jobs, nfill, hwe_sb, hue_sb, wb_sb, wf_sb, Sp_d,
               opool, rpool, fpool_d, fpool_a, fpool_p, pspool_s, pspool_j):
    if nfill:
        junk = pspool_j.tile([P, L], F32, tag="junk")
        for _ in range(nfill):
            nc.tensor.matmul(junk[:], lhsT=hwe_sb["d"][0][:, 0:P],
                             rhs=hwe_sb["d"][0][:], start=True, stop=True)

    S_ps = pspool_s.tile([ACP, L], F32, tag="S")
    nmm = sum(2 if t.endswith("b") else 1 for t in jobs)
    idx = 0
    kb = kf = 0
    def emit_relu(eng, out_ap, half, j):
        hw = hwe_sb[eng][half]
        hu = hue_sb[eng][half]
        if eng == "d":
            nc.vector.tensor_scalar(
                out_ap, hw[:], hu[:, j:j + 1], 0.0,
                mybir.AluOpType.add, mybir.AluOpType.max)
        elif eng == "a":
            nc.scalar.activation(
                out_ap, hw[:],
                mybir.ActivationFunctionType.Relu,
                bias=hu[:, j:j + 1], scale=1.0)
        else:
            nc.gpsimd.tensor_scalar(
                out_ap, hw[:], hu[:, j:j + 1], 0.0,
                mybir.AluOpType.add, mybir.AluOpType.max)

    bpools = {"d": rpool, "a": fpool_a, "p": fpool_p}
    fpools = {"d": fpool_d, "a": fpool_a, "p": fpool_p}
    for pos, t in enumerate(jobs):
        j = pos   # job index == local j index
        eng, dt = t[0], t[1]
        if dt == "b":
            ck, jj = divmod(kb, CB)
            kb += 1
            for half in range(2):
                r = bpools[eng].tile([P, L], BF16, tag=f"r{eng}")
                emit_relu(eng, r[:], half, j)
                nc.tensor.matmul(S_ps[:], lhsT=wb_sb[ck][:, jj, half],
                                 rhs=r[:], start=(idx == 0),
                                 stop=(idx == nmm - 1))
                idx += 1
        else:
            ck, jj = divmod(kf, CF)
            kf += 1
            pr = fpools[eng].tile([P, 2, L], FP8, tag=f"pr{eng}")
            for half in range(2):
                emit_relu(eng, pr[:, half], half, j)
            nc.tensor.matmul(S_ps[:], lhsT=wf_sb[ck][:, jj], rhs=pr[:],
                             start=(idx == 0), stop=(idx == nmm - 1),
                             perf_mode=mybir.MatmulPerfMode.DoubleRow)
            idx += 1

    # Tail: single ACT copy (DVE is the pacer; ACT has slack) + one DMA.
    out_sb = opool.tile([AC, L], F32, tag="out")
    nc.scalar.copy(out_sb[:], S_ps[0:AC, :])
    nc.sync.dma_start(Sp_d[:], out_sb[:])


def _quant_ef(Wseq):
    """Error-feedback fp8 quantization along axis 0 (the j sequence)."""
    e4 = ml_dtypes.float8_e4m3
    out = np.empty(Wseq.shape, e4)
    carry = np.zeros(Wseq.shape[1:], np.float32)
    for k in range(Wseq.shape[0]):
        t = Wseq[k] + carry
        q = t.astype(e4)
        carry = t - q.astype(np.float32)
        out[k] = q
    return out


def _prepare_in_maps(h, X, mask, Wx_w, Wx_b, Ux_w, Ux_b, Tx_w, Tx_b,
                     split=None):
    jobs = assign_jobs(split)
    bsel = np.array([t.endswith("b") for t in jobs])
    m = mask.astype(np.float32)                                   # (B, L)
    hwp = (h.astype(np.float32) @ Wx_w.T.astype(np.float32)
           + Wx_b.astype(np.float32))                             # (B, L, H)
    hup = (h.astype(np.float32) @ Ux_w.T.astype(np.float32)
           + Ux_b.astype(np.float32))
    hwT = np.ascontiguousarray(hwp.transpose(0, 2, 1)).astype(
        ml_dtypes.bfloat16)                                       # (B, H, L)
    huT = np.ascontiguousarray(hup.transpose(0, 2, 1)).astype(np.float32)

    Xm4 = np.empty((B, L, A, 4), np.float32)
    Xm4[..., :3] = X * m[:, :, None, None]
    Xm4[..., 3] = m[:, :, None]

    TxT_ac = np.repeat(Tx_w.T.astype(np.float32), 4, axis=1)      # (H, 56)
    Xm4_ac = Xm4.reshape(B, L, AC)

    def build_core(c):
        b, q = divmod(c, 4)
        j0 = q * JSHARD
        Wc = (TxT_ac[None, :, :] * Xm4_ac[b, j0:j0 + JSHARD, None, :])
        Wc4 = Wc.reshape(JSHARD, 2, P, AC)      # [j, half, hh, ac]
        Wb_j = Wc4[bsel]
        Wf_j = Wc4[~bsel]
        nb = Wb_j.shape[0]
        nf = Wf_j.shape[0]
        if nb:
            Wbp = np.zeros((nb, 2, P, ACP), np.float32)
            Wbp[..., :AC] = Wb_j
            Wb = np.ascontiguousarray(
                Wbp.transpose(2, 0, 1, 3).reshape(P, nb * 2 * ACP)
            ).astype(ml_dtypes.bfloat16)
        else:
            Wb = np.zeros((P, 2 * ACP), ml_dtypes.bfloat16)
        if nf:
            Wfq = _quant_ef(Wf_j.astype(np.float32))
            Wfp = np.zeros((nf, 2, P, ACP), ml_dtypes.float8_e4m3)
            Wfp[..., :AC] = Wfq
            Wf = np.ascontiguousarray(
                Wfp.transpose(2, 0, 1, 3).reshape(P, nf * 2 * ACP))
        else:
            Wf = np.zeros((P, 2 * ACP), ml_dtypes.float8_e4m3)
        hw_c = np.ascontiguousarray(hwT[b].reshape(2, P, L))
        hu_c = np.ascontiguousarray(
            huT[b, :, j0:j0 + JSHARD].reshape(2, P, JSHARD))
        return {"hw": hw_c, "hu": hu_c, "Wb": Wb, "Wf": Wf}

    with ThreadPoolExecutor(max_workers=NCORES) as ex:
        in_maps = list(ex.map(build_core, range(NCORES)))
    return in_maps, m, Xm4


def _epilogue(results, X, m, Xm4, Tx_b):
    S4 = np.zeros((B, A, 4, L), np.float32)
    for c in range(NCORES):
        S4[c // 4] += results[c]["Sp"].reshape(A, 4, L)
    Sraw = S4.transpose(0, 3, 1, 2)                               # (B, L, A, 4)
    CX = Xm4.sum(axis=1)                                          # (B, A, 4)
    S_tot = Sraw + Tx_b[None, None, :, None] * CX[:, None]        # (B, L, A, 4)
    G = S_tot[..., 3]                                             # (B, L, A)
    S3 = S_tot[..., :3]                                           # (B, L, A, 3)
    denom = 1e-6 + m.sum(axis=1)[:, None, None, None]
    f = (X * G[..., None] - S3) / denom
    return (X + np.clip(f, -20.0, 20.0)).astype(np.float32)


def _run(trace=False, **inputs):
    inputs = {k: np.asarray(v) for k, v in inputs.items()}
    X = inputs["X"].astype(np.float32)
    nc = _build_program()
    in_maps, m, Xm4 = _prepare_in_maps(**inputs)
    for attempt in range(3):
        res = run_bass_kernel_spmd(nc, in_maps, core_ids=list(range(NCORES)),
                                   trace=trace)
        if all(np.isfinite(r["Sp"]).all() for r in res.results):
            break
    out = _epilogue(res.results, X, m, Xm4, inputs["Tx_b"].astype(np.float32))
    return out, res


def kernel(**inputs):
    out, _ = _run(trace=False, **inputs)
    return out


# revision 16
# speedup vs baseline: 1.1773x; 1.1773x over previous
"""Fused FADocker coordinate-update kernel for 8 Trainium2 NeuronCores (v3).

Per core c: batch b=c//4, j-range (c%4)*96..+96,
    S'[(a,c'), i] = sum_{j,h} W_j[h,(a,c')] * relu(hwT[h,i] + huT[h,j])
with hw = W_x(h), hu = U_x(h) projections and W_j[h,ac] = TxT[h,a]*Xm4[j,ac]
precomputed on host. Host sums the 8 partials and applies the Tx_b
correction, divide, clip, residual add.

Engine plan (cost-model-driven): H=256 contracts as two 128-row halves.
Each j is a pair job:
  - bf16 (DVE): two bf16 relu tiles (160ns each, 4x DVE mode) -> two bf16
    matmuls (160ns at full PE p-state).
  - fp8 (DVE 260ns/half, ACT 505, Pool 628): both halves of a [128,2,384]
    fp8e4 pair tile -> ONE DoubleRow matmul (80ns) contracting both halves.
fp8 W is error-feedback quantized over j on host so the j-sum telescopes
quantization error (rel err ~3e-3 vs 2.5e-2 naive). All four engines are
balanced near 20.5us; consts load outside the timing loop and PSUM/out
tiles are double-buffered so loop iterations pipeline.
"""

import contextlib
from concurrent.futures import ThreadPoolExecutor

import numpy as np
import ml_dtypes

import concourse.bass as bass
import concourse.tile as tile
from concourse import bacc, mybir
from concourse.bass import RegisterHandles, make_scalar_value
from concourse.bass_utils import run_bass_kernel_spmd

B, L, H, A = 2, 384, 256, 14
NCORES = 8
JSHARD = L // 4          # 96 j's per core
AC = A * 4               # 56 = (a, c') columns
ACP = 64                 # AC padded to 64 (DoubleRow needs k-tile stride %16==0)
P = 128

F32 = mybir.dt.float32
BF16 = mybir.dt.bfloat16
FP8 = mybir.dt.float8e4

# pairs per (engine, dtype): db/df = DVE bf16/fp8, ab/af = ACT, pb/pf = Pool;
# sum = 96
SPLIT = (65, 0, 0, 31, 0, 0)
NFILL = 0                # PE warm-up filler matmuls (first-iteration only aid)
CB = 16                  # bf16 W chunk: j's per DMA
CF = 24                  # fp8 W chunk: j's per DMA

_cached = {}


def assign_jobs(split=None, nclust=4):
    """Per-job engine tags (db/df/ab/af/pb/pf). PE mode switches
    (plain<->DoubleRow) cost ~280ns, but one big DR block serializes after
    the DVE-paced bf16 stream; compromise: nclust clusters, each
    [bf16 chunk, DR chunk], so switches stay few AND the DR work
    interleaves into PE's slack."""
    split = split or SPLIT
    bf = []
    f8 = []
    for tag, n in zip(("db", "ab", "pb", "df", "af", "pf"), (
            split[0], split[2], split[4], split[1], split[3], split[5])):
        lst = bf if tag.endswith("b") else f8
        lst += [tag] * n
    jobs = []
    for c in range(nclust):
        jobs += bf[len(bf) * c // nclust:len(bf) * (c + 1) // nclust]
        jobs += f8[len(f8) * c // nclust:len(f8) * (c + 1) // nclust]
    return jobs


def _build_program(reps=1, dyn_loop=False, split=None, nfill=None):
    split = split or SPLIT
    nfill = NFILL if nfill is None else nfill
    key = ("nc", reps, dyn_loop, split, nfill)
    if key in _cached:
        return _cached[key]

    jobs = assign_jobs(split)
    nb = sum(1 for t in jobs if t.endswith("b"))
    nf = len(jobs) - nb
    nb_chunks = max(1, (nb + CB - 1) // CB)
    nf_chunks = max(1, (nf + CF - 1) // CF)

    nc = bacc.Bacc("TRN2", target_bir_lowering=False, debug=False)

    hw_d = nc.dram_tensor("hw", [2, P, L], BF16, kind="ExternalInput").ap()
    hu_d = nc.dram_tensor("hu", [2, P, JSHARD], F32,
                          kind="ExternalInput").ap()
    Wb_d = nc.dram_tensor("Wb", [P, max(nb, 1) * 2 * ACP], BF16,
                          kind="ExternalInput").ap()
    Wf_d = nc.dram_tensor("Wf", [P, max(nf, 1) * 2 * ACP], FP8,
                          kind="ExternalInput").ap()
    if dyn_loop:
        ln_d = nc.dram_tensor("ln", [1, 1], mybir.dt.int32,
                              kind="ExternalInput").ap()
    Sp_d = nc.dram_tensor("Sp", [AC, L], F32, kind="ExternalOutput").ap()

    with tile.TileContext(nc, trace_sim=False) as tc:
        with (
            tc.tile_pool(name="const", bufs=1) as cpool,
            tc.tile_pool(name="outp", bufs=2) as opool,
            tc.tile_pool(name="rb", bufs=48) as rpool,
            tc.tile_pool(name="fd", bufs=8) as fpool_d,
            tc.tile_pool(name="fa", bufs=70) as fpool_a,
            tc.tile_pool(name="fp", bufs=4) as fpool_p,
            tc.tile_pool(name="psum_s", bufs=2, space="PSUM") as pspool_s,
            tc.tile_pool(name="psum_j", bufs=1, space="PSUM") as pspool_j,
        ):
            # Input tiles: hw/hu first (gate the producers), W chunks in
            # first-use order alternating between the sync HWDGE queue and
            # the scalar SWDGE queue.
            hwe_sb = {}
            hue_sb = {}
            for e in "dap":
                hwe_sb[e] = []
                hue_sb[e] = []
                for k in range(2):
                    eng = nc.sync if (k % 2 == 0) else nc.scalar
                    t = cpool.tile([P, L], BF16, tag=f"hw{e}{k}")
                    eng.dma_start(t[:], hw_d[k])
                    hwe_sb[e].append(t)
                    t2 = cpool.tile([P, JSHARD], F32, tag=f"hu{e}{k}")
                    eng.dma_start(t2[:], hu_d[k])
                    hue_sb[e].append(t2)
            hwT_sb = hwe_sb["d"]
            huT_sb = hue_sb["d"]

            buse = {}
            fuse = {}
            kb = kf = 0
            for pos, t in enumerate(jobs):
                if t.endswith("b"):
                    buse.setdefault(kb // CB, pos)
                    kb += 1
                else:
                    fuse.setdefault(kf // CF, pos)
                    kf += 1
            order = sorted(
                [("b", ck, p) for ck, p in buse.items()]
                + [("f", ck, p) for ck, p in fuse.items()], key=lambda x: x[2])
            wb_sb = [None] * nb_chunks
            wf_sb = [None] * nf_chunks
            for i, (kind, ck, _) in enumerate(order):
                eng = nc.sync if i % 2 == 0 else nc.scalar
                if kind == "b":
                    n_j = min(CB, nb - ck * CB)
                    t = cpool.tile([P, n_j, 2, ACP], BF16, tag=f"wb{ck}")
                    c0 = ck * CB * 2 * ACP
                    eng.dma_start(t[:], Wb_d[:, c0:c0 + n_j * 2 * ACP])
                    wb_sb[ck] = t
                else:
                    n_j = min(CF, nf - ck * CF)
                    t = cpool.tile([P, n_j, 2, ACP], FP8, tag=f"wf{ck}")
                    c0 = ck * CF * 2 * ACP
                    eng.dma_start(t[:], Wf_d[:, c0:c0 + n_j * 2 * ACP])
                    wf_sb[ck] = t

            if dyn_loop:
                ln_t = cpool.tile([1, 1], mybir.dt.int32, tag="ln")
                nc.sync.dma_start(ln_t[:], ln_d[:])
                regs = []
                for e in mybir.ALL_ENGINES:
                    r = nc.alloc_register(e, f"lnreg_{e.name}")
                    nc.engines[e].reg_load(r, ln_t[0:1, 0:1])
                    regs.append(r)
                end_val = make_scalar_value(RegisterHandles(regs),
                                            min_val=0, max_val=1 << 20)
                loop_cm = tc.For_i(0, end_val, 1)
            else:
                loop_cm = contextlib.nullcontext()
            with loop_cm:
                for rep in range(reps):
                    _emit_body(nc, tc, jobs, nfill, hwe_sb, hue_sb, wb_sb,
                               wf_sb, Sp_d, opool, rpool, fpool_d, fpool_a,
                               fpool_p, pspool_s, pspool_j)

    nc.compile()
    _cached[key] = nc
    return nc


def _emit_body(nc, tc, jobs, nfill, hwe_sb, hue_sb, wb_sb, wf_sb, Sp_d,
               opool, rpool, fpool_d, fpool_a, fpool_p, pspool_s, pspool_j):
    if nfill:
        junk = pspool_j.tile([P, L], F32, tag="junk")
        for _ in range(nfill):
            nc.tensor.matmul(junk[:], lhsT=hwe_sb["d"][0][:, 0:P],
                             rhs=hwe_sb["d"][0][:], start=True, stop=True)

    S_ps = pspool_s.tile([ACP, L], F32, tag="S")
    nmm = sum(2 if t.endswith("b") else 1 for t in jobs)
    idx = 0
    kb = kf = 0
    def emit_relu(eng, out_ap, half, j):
        hw = hwe_sb[eng][half]
        hu = hue_sb[eng][half]
        if eng == "d":
            nc.vector.tensor_scalar(
                out_ap, hw[:], hu[:, j:j + 1], 0.0,
                mybir.AluOpType.add, mybir.AluOpType.max)
        elif eng == "a":
            nc.scalar.activation(
                out_ap, hw[:],
                mybir.ActivationFunctionType.Relu,
                bias=hu[:, j:j + 1], scale=1.0)
        else:
            nc.gpsimd.tensor_scalar(
                out_ap, hw[:], hu[:, j:j + 1], 0.0,
                mybir.AluOpType.add, mybir.AluOpType.max)

    bpools = {"d": rpool, "a": fpool_a, "p": fpool_p}
    fpools = {"d": fpool_d, "a": fpool_a, "p": fpool_p}
    for pos, t in enumerate(jobs):
        j = pos   # job index == local j index
        eng, dt = t[0], t[1]
        if dt == "b":
            ck, jj = divmod(kb, CB)
            kb += 1
            for half in range(2):
                r = bpools[eng].tile([P, L], BF16, tag=f"r{eng}")
                emit_relu(eng, r[:], half, j)
                nc.tensor.matmul(S_ps[:], lhsT=wb_sb[ck][:, jj, half],
                                 rhs=r[:], start=(idx == 0),
                                 stop=(idx == nmm - 1))
                idx += 1
        else:
            ck, jj = divmod(kf, CF)
            kf += 1
            pr = fpools[eng].tile([P, 2, L], FP8, tag=f"pr{eng}")
            for half in range(2):
                emit_relu(eng, pr[:, half], half, j)
            nc.tensor.matmul(S_ps[:], lhsT=wf_sb[ck][:, jj], rhs=pr[:],
                             start=(idx == 0), stop=(idx == nmm - 1),
                             perf_mode=mybir.MatmulPerfMode.DoubleRow)
            idx += 1

    # Tail: single ACT copy (DVE is the pacer; ACT has slack) + one DMA.
    out_sb = opool.tile([AC, L], F32, tag="out")
    nc.scalar.copy(out_sb[:], S_ps[0:AC, :])
    nc.sync.dma_start(Sp_d[:], out_sb[:])


def _quant_ef(Wseq):
    """Error-feedback fp8 quantization along axis 0 (the j sequence)."""
    e4 = ml_dtypes.float8_e4m3
    out = np.empty(Wseq.shape, e4)
    carry = np.zeros(Wseq.shape[1:], np.float32)
    for k in range(Wseq.shape[0]):
        t = Wseq[k] + carry
        q = t.astype(e4)
        carry = t - q.astype(np.float32)
        out[k] = q
    return out


def _prepare_in_maps(h, X, mask, Wx_w, Wx_b, Ux_w, Ux_b, Tx_w, Tx_b,
                     split=None):
    jobs = assign_jobs(split)
    bsel = np.array([t.endswith("b") for t in jobs])
    m = mask.astype(np.float32)                                   # (B, L)
    hwp = (h.astype(np.float32) @ Wx_w.T.astype(np.float32)
           + Wx_b.astype(np.float32))                             # (B, L, H)
    hup = (h.astype(np.float32) @ Ux_w.T.astype(np.float32)
           + Ux_b.astype(np.float32))
    hwT = np.ascontiguousarray(hwp.transpose(0, 2, 1)).astype(
        ml_dtypes.bfloat16)                                       # (B, H, L)
    huT = np.ascontiguousarray(hup.transpose(0, 2, 1)).astype(np.float32)

    Xm4 = np.empty((B, L, A, 4), np.float32)
    Xm4[..., :3] = X * m[:, :, None, None]
    Xm4[..., 3] = m[:, :, None]

    TxT_ac = np.repeat(Tx_w.T.astype(np.float32), 4, axis=1)      # (H, 56)
    Xm4_ac = Xm4.reshape(B, L, AC)

    def build_core(c):
        b, q = divmod(c, 4)
        j0 = q * JSHARD
        Wc = (TxT_ac[None, :, :] * Xm4_ac[b, j0:j0 + JSHARD, None, :])
        Wc4 = Wc.reshape(JSHARD, 2, P, AC)      # [j, half, hh, ac]
        Wb_j = Wc4[bsel]
        Wf_j = Wc4[~bsel]
        nb = Wb_j.shape[0]
        nf = Wf_j.shape[0]
        if nb:
            Wbp = np.zeros((nb, 2, P, ACP), np.float32)
            Wbp[..., :AC] = Wb_j
            Wb = np.ascontiguousarray(
                Wbp.transpose(2, 0, 1, 3).reshape(P, nb * 2 * ACP)
            ).astype(ml_dtypes.bfloat16)
        else:
            Wb = np.zeros((P, 2 * ACP), ml_dtypes.bfloat16)
        if nf:
            Wfq = _quant_ef(Wf_j.astype(np.float32))
            Wfp = np.zeros((nf, 2, P, ACP), ml_dtypes.float8_e4m3)
            Wfp[..., :AC] = Wfq
            Wf = np.ascontiguousarray(
                Wfp.transpose(2, 0, 1, 3).reshape(P, nf * 2 * ACP))
        else:
            Wf = np.zeros((P, 2 * ACP), ml_dtypes.float8_e4m3)
        hw_c = np.ascontiguousarray(hwT[b].reshape(2, P, L))
        hu_c = np.ascontiguousarray(
            huT[b, :, j0:j0 + JSHARD].reshape(2, P, JSHARD))
        return {"hw": hw_c, "hu": hu_c, "Wb": Wb, "Wf": Wf}

    with ThreadPoolExecutor(max_workers=NCORES) as ex:
        in_maps = list(ex.map(build_core, range(NCORES)))
    return in_maps, m, Xm4


def _epilogue(results, X, m, Xm4, Tx_b):
    S4 = np.zeros((B, A, 4, L), np.float32)
    for c in range(NCORES):
        S4[c // 4] += results[c]["Sp"].reshape(A, 4, L)
    Sraw = S4.transpose(0, 3, 1, 2)                               # (B, L, A, 4)
    CX = Xm4.sum(axis=1)                                          # (B, A, 4)
    S_tot = Sraw + Tx_b[None, None, :, None] * CX[:, None]        # (B, L, A, 4)
    G = S_tot[..., 3]                                             # (B, L, A)
    S3 = S_tot[..., :3]                                           # (B, L, A, 3)
    denom = 1e-6 + m.sum(axis=1)[:, None, None, None]
    f = (X * G[..., None] - S3) / denom
    return (X + np.clip(f, -20.0, 20.0)).astype(np.float32)


def _run(trace=False, **inputs):
    inputs = {k: np.asarray(v) for k, v in inputs.items()}
    X = inputs["X"].astype(np.float32)
    nc = _build_program()
    in_maps, m, Xm4 = _prepare_in_maps(**inputs)
    for attempt in range(3):
        res = run_bass_kernel_spmd(nc, in_maps, core_ids=list(range(NCORES)),
                                   trace=trace)
        if all(np.isfinite(r["Sp"]).all() for r in res.results):
            break
    out = _epilogue(res.results, X, m, Xm4, inputs["Tx_b"].astype(np.float32))
    return out, res


def kernel(**inputs):
    out, _ = _run(trace=False, **inputs)
    return out
